# revision 10
# baseline (speedup 1.0000x reference)
"""Multi-head causal attention with interleaved RoPE on 8 Trainium2 cores.

nn_MultiHeadAttention: x[4,2048,1024], W_qkv[3072,1024], W_o[1024,1024],
16 heads x d_k=64, interleaved RoPE, causal softmax.

Sharding: core c = 2*b + g handles batch b (of 4) and head-group g (of 2,
8 heads each). Each core computes a full-width partial output for its batch
(o_heads @ W_o[:, group-cols]); the host sums the two partials per batch
(the "all-reduce after o_proj", done on host at gather time).

Device strategy (per core):
 - host passes x[b] transposed (xT [1024,2048]) and W slices transposed, with
   q/k rows permuted even-first so interleaved RoPE becomes rotate-half.
 - fp32r matmuls everywhere (1 cyc/row vs 4 for fp32 at moving dim >= 256).
 - QKV proj on PE, out q^T/k^T in [head_dim, seq] layout; RoPE applied with
   a gpsimd 32-row swap + DVE mul/mul/add against host-built cos/sin tables
   (sign of sin baked into the table rows).
 - scores computed transposed: S^T[k,q] = k_rot . q_rot per head; exp on ACT
   (1/sqrt(dk) fused into the activation scale; no max-subtraction needed:
   scores are O(15) max, fp32-safe); causal = block skipping + one additive
   -1e30 mask on the 128x128 diagonal block before exp.
 - PV with lhsT = [v | ones]: the softmax denominator falls out as row 64 of
   the PSUM accumulator; normalize after PV, directly producing o^T which is
   exactly the lhsT that o_proj needs. No transposes of P or o anywhere.
"""

import numpy as np
from contextlib import ExitStack

NUM_HEADS = 16
D_K = 64
THETA = 10000.0
BS, S, D = 4, 2048, 1024
N_CORES = 8
HPC = NUM_HEADS // 2          # heads per core = 8
DG = HPC * D_K                # per-core head width = 512
QT2 = 1024                    # q tile (2 PSUM banks)

USE_F32R = True

_compiled = None


def _build_program(stop_after=None):
    import concourse.bass as bass
    import concourse.mybir as mybir
    import concourse.tile as tile
    from concourse import bacc

    F32 = mybir.dt.float32
    FR = mybir.dt.float32r if USE_F32R else mybir.dt.float32
    AF = mybir.ActivationFunctionType

    nc = bacc.Bacc("TRN2", target_bir_lowering=False, debug=False,
                   num_devices=N_CORES)

    xt_d = nc.dram_tensor("xt", [D, S], F32, kind="ExternalInput")
    wqkvt_d = nc.dram_tensor("wqkvt", [D, 3 * DG], F32, kind="ExternalInput")
    wot_d = nc.dram_tensor("wot", [DG, D], F32, kind="ExternalInput")
    cos_d = nc.dram_tensor("cost", [128, S], F32, kind="ExternalInput")
    sin_d = nc.dram_tensor("sint", [128, S], F32, kind="ExternalInput")
    out_d = nc.dram_tensor("out", [S, D], F32, kind="ExternalOutput")

    n_sb = S // 128           # 16 s-blocks
    n_st = S // 512           # 4 s-tiles
    n_db = D // 128           # 8 d-blocks
    inv_sqrt_dk = 1.0 / float(np.sqrt(D_K))

    def load_xt_strip(pool, st):
        """One gpsimd DMA (fp32 -> fp32r cast): [128, 8*512] strip tile where
        cols db*512:(db+1)*512 hold xT[db*128:(db+1)*128, st*512:(st+1)*512]."""
        t = pool.tile([128, n_db * 512], FR, tag="xts", name="xts")
        for db in range(n_db):
            nc.gpsimd.dma_start(
                t[:, db * 512:(db + 1) * 512],
                xt_d.ap()[db * 128:(db + 1) * 128,
                          st * 512:(st + 1) * 512])
        return t

    with tile.TileContext(nc) as tc, ExitStack() as octx:
        OP = octx.enter_context
        # ---------- persistent pools (whole kernel) ----------
        qk_p = OP(tc.tile_pool(name="qk", bufs=1))
        sps_p = OP(tc.tile_pool(name="sps", bufs=2, space="PSUM"))
        ops_p = OP(tc.tile_pool(name="ops", bufs=2, space="PSUM"))

        # q_rot/k_rot: [512 e, 2048 s] as 4 tiles of [128, S] each
        qrot = [qk_p.tile([128, S], FR, tag=f"qrot{i}", name=f"qrot{i}")
                for i in range(4)]
        krot = [qk_p.tile([128, S], FR, tag=f"krot{i}", name=f"krot{i}")
                for i in range(4)]

        # ============ Phase P1: q/k projection + RoPE ============
        with ExitStack() as ctx:
            P = ctx.enter_context
            cs_p = P(tc.tile_pool(name="cs", bufs=1))
            xt_p = P(tc.tile_pool(name="xtp", bufs=2))
            w_p = P(tc.tile_pool(name="w", bufs=1))
            tmp_p = P(tc.tile_pool(name="tmp", bufs=1))
            rot_p = P(tc.tile_pool(name="rot", bufs=2))

            cos_t = cs_p.tile([128, S], F32)
            sin_t = cs_p.tile([128, S], F32)
            nc.sync.dma_start(cos_t[:], cos_d.ap())
            nc.sync.dma_start(sin_t[:], sin_d.ap())

            # W_qk^T resident: 8 tiles [128 d, 1024 e]
            wqk = [w_p.tile([128, 2 * DG], FR, tag=f"wqk{i}", name=f"wqk{i}")
                   for i in range(n_db)]
            for db in range(n_db):
                nc.gpsimd.dma_start(
                    wqk[db][:], wqkvt_d.ap()[db * 128:(db + 1) * 128, 0:1024])

            for st in range(n_st):
                xts = load_xt_strip(xt_p, st)
                sl = slice(st * 512, (st + 1) * 512)
                for eb in range(8):
                    dst = qrot[eb] if eb < 4 else krot[eb - 4]
                    esl = slice(eb * 128, (eb + 1) * 128)
                    ps = sps_p.tile([128, 512], F32, tag="sc")
                    for db in range(n_db):
                        nc.tensor.matmul(
                            ps[:], wqk[db][:, esl],
                            xts[:, db * 512:(db + 1) * 512],
                            start=(db == 0), stop=(db == n_db - 1))
                    # RoPE: dst = ps*cos + swap32(ps)*sin_signed
                    t1 = rot_p.tile([128, 512], F32, tag="t1")
                    nc.vector.tensor_mul(t1[:], ps[:], cos_t[:, sl])
                    qtmp = tmp_p.tile([128, 512], F32, tag="qtmp")
                    nc.scalar.copy(qtmp[:], ps[:])
                    qsw = tmp_p.tile([128, 512], F32, tag="qsw")
                    for g in range(2):
                        a, b = g * 64, g * 64 + 32
                        nc.gpsimd.tensor_copy(qsw[a:a + 32, :],
                                              qtmp[b:b + 32, :])
                        nc.gpsimd.tensor_copy(qsw[b:b + 32, :],
                                              qtmp[a:a + 32, :])
                    t2 = rot_p.tile([128, 512], F32, tag="t2")
                    nc.vector.tensor_mul(t2[:], qsw[:], sin_t[:, sl])
                    nc.vector.tensor_add(dst[:, sl], t1[:], t2[:])

        if stop_after == "p1":
            with ExitStack() as ctx:
                op_ = ctx.enter_context(tc.tile_pool(name="dumo", bufs=1))
                for i in range(4):
                    nc.sync.dma_start(
                        out_d.ap()[i * 128:(i + 1) * 128, :],
                        qrot[i][:, 0:D].bitcast(mybir.dt.float32))
            nc.compile()
            return nc

        # ============ Phase P2: v projection into [v | ones] ============
        vaug_p = OP(tc.tile_pool(name="vaug", bufs=1))
        vaug = [vaug_p.tile([128, HPC * (D_K + 1)], FR, tag=f"va{i}",
                        name=f"va{i}") for i in range(n_sb)]
        with ExitStack() as ctx:
            P = ctx.enter_context
            xt_p = P(tc.tile_pool(name="xtp2", bufs=2))
            wv_p = P(tc.tile_pool(name="wv", bufs=1))
            ones_t = wv_p.tile([128, HPC], F32, name="ones_t")
            nc.gpsimd.memset(ones_t[:], 1.0)
            wv = [wv_p.tile([128, DG], FR, tag=f"wv{i}", name=f"wv{i}")
                  for i in range(n_db)]
            for db in range(n_db):
                nc.gpsimd.dma_start(
                    wv[db][:],
                    wqkvt_d.ap()[db * 128:(db + 1) * 128, 1024:1536])
            for st in range(n_st):
                xts = load_xt_strip(xt_p, st)
                for j in range(4):
                    sb = st * 4 + j
                    ps = sps_p.tile([128, 512], F32, tag="sc")
                    for db in range(n_db):
                        nc.tensor.matmul(
                            ps[:],
                            xts[:, db * 512 + j * 128:db * 512 + (j + 1) * 128],
                            wv[db][:],
                            start=(db == 0), stop=(db == n_db - 1))
                    src = ps[:].rearrange("p (h c) -> p h c", c=D_K)
                    dst = vaug[sb][:].rearrange("p (h c) -> p h c", c=D_K + 1)
                    nc.vector.tensor_copy(dst[:, :, 0:D_K], src)
                    nc.vector.tensor_copy(
                        dst[:, :, D_K:D_K + 1],
                        ones_t[:].rearrange("p (h c) -> p h c", c=1))

        if stop_after == "p2":
            for i in range(4):
                nc.sync.dma_start(
                    out_d.ap()[i * 128:(i + 1) * 128, :],
                    vaug[i][:, 0:520].bitcast(mybir.dt.float32)[:, 0:512].rearrange("p n -> p n"))
            nc.compile()
            return nc

        # ============ Phase A: attention ============
        ot_p = OP(tc.tile_pool(name="ot", bufs=1))
        ot = [ot_p.tile([128, S], FR, tag=f"ot{i}", name=f"oti{i}")
              for i in range(4)]
        wot_p = OP(tc.tile_pool(name="wot", bufs=1))
        wot = [wot_p.tile([128, D], FR, tag=f"wot{i}", name=f"wott{i}")
               for i in range(4)]
        for t in range(4):
            nc.gpsimd.dma_start(wot[t][:],
                                wot_d.ap()[t * 128:(t + 1) * 128, :])

        with ExitStack() as ctx:
            P = ctx.enter_context
            const_p = P(tc.tile_pool(name="amisc", bufs=1))
            pt_p = P(tc.tile_pool(name="pt", bufs=3))
            nrm_p = P(tc.tile_pool(name="nrm", bufs=2))

            # additive causal mask for the S^T diagonal block: 0 where k <= q,
            # -1e30 where k > q
            dmask = const_p.tile([128, 128], F32)
            nc.gpsimd.memset(dmask[:], 0.0)
            nc.gpsimd.affine_select(
                out=dmask[:], in_=dmask[:],
                compare_op=mybir.AluOpType.is_ge, fill=-1e30, base=0,
                pattern=[[1, 128]], channel_multiplier=-1,
            )

            for h in range(HPC):
                ti, po = h // 2, (h % 2) * 64
                vlo = h * (D_K + 1)
                for q2 in range(S // QT2):
                    q0 = q2 * QT2
                    kb_end = (q0 + QT2) // 128
                    kb_last0 = q0 // 128 + 3      # last kb writing bank 0
                    ops = ops_p.tile([D_K + 1, QT2], F32, tag="ot")
                    for kb in range(kb_end):
                        c0 = max(0, kb * 128 - q0)
                        sc = sps_p.tile([128, QT2], F32, tag="sc")
                        # scores S^T (k on partitions, q on free), per bank
                        if c0 < 512:
                            nc.tensor.matmul(
                                sc[:, c0:512],
                                krot[ti][po:po + 64, kb * 128:(kb + 1) * 128],
                                qrot[ti][po:po + 64, q0 + c0:q0 + 512],
                                start=True, stop=True)
                        b1 = max(c0, 512)
                        nc.tensor.matmul(
                            sc[:, b1:QT2],
                            krot[ti][po:po + 64, kb * 128:(kb + 1) * 128],
                            qrot[ti][po:po + 64, q0 + b1:q0 + QT2],
                            start=True, stop=True)
                        # causal diagonal: additive -1e30 pre-exp
                        if kb * 128 >= q0:
                            nc.vector.tensor_add(sc[:, c0:c0 + 128],
                                                 sc[:, c0:c0 + 128], dmask[:])
                        pt = pt_p.tile([128, QT2], FR, tag="pt")
                        nc.scalar.activation(pt[:, c0:QT2], sc[:, c0:QT2],
                                             AF.Exp, scale=inv_sqrt_dk)
                        # PV (+ ones row -> denominator lands in row 64)
                        if c0 < 512:
                            nc.tensor.matmul(
                                ops[:, c0:512],
                                vaug[kb][:, vlo:vlo + D_K + 1],
                                pt[:, c0:512],
                                start=(kb == 0), stop=(kb == kb_last0))
                        nc.tensor.matmul(
                            ops[:, b1:QT2],
                            vaug[kb][:, vlo:vlo + D_K + 1],
                            pt[:, b1:QT2],
                            start=(kb == 0), stop=(kb == kb_end - 1))
                    # normalize: o^T[dv, q] * (1/den[q])
                    r64 = nrm_p.tile([65, QT2], F32, tag="r64")
                    nc.vector.tensor_copy(r64[64:65, :], ops[D_K:D_K + 1, :])
                    r0 = nrm_p.tile([1, QT2], F32, tag="r0")
                    nc.gpsimd.tensor_copy(r0[:], r64[64:65, :])
                    rinv = nrm_p.tile([1, QT2], F32, tag="rinv")
                    nc.vector.reciprocal(rinv[:], r0[:])
                    den = nrm_p.tile([64, QT2], F32, tag="den")
                    nc.gpsimd.partition_broadcast(den[:], rinv[:])
                    if po == 0:
                        nc.vector.tensor_mul(ot[ti][0:64, q0:q0 + QT2],
                                             ops[0:D_K, :], den[:])
                    else:
                        onrm = nrm_p.tile([64, QT2], FR, tag="onrm")
                        nc.vector.tensor_mul(onrm[:], ops[0:D_K, :], den[:])
                        nc.gpsimd.tensor_copy(ot[ti][64:128, q0:q0 + QT2],
                                              onrm[:])

        if stop_after == "a":
            for i in range(4):
                nc.sync.dma_start(
                    out_d.ap()[i * 128:(i + 1) * 128, :],
                    ot[i][:, 0:D].bitcast(mybir.dt.float32))
            nc.compile()
            return nc

        # ============ Phase O: o_proj ============
        with ExitStack() as ctx:
            P = ctx.enter_context
            outs_p = P(tc.tile_pool(name="outs", bufs=2))
            for sb in range(n_sb):
                ssl = slice(sb * 128, (sb + 1) * 128)
                ostage = outs_p.tile([128, D], F32, tag="ostage")
                for eh in range(2):
                    esl = slice(eh * 512, (eh + 1) * 512)
                    ps = sps_p.tile([128, 512], F32, tag="sc")
                    for t in range(4):
                        nc.tensor.matmul(ps[:], ot[t][:, ssl], wot[t][:, esl],
                                         start=(t == 0), stop=(t == 3))
                    nc.scalar.copy(ostage[:, esl], ps[:])
                nc.sync.dma_start(out_d.ap()[ssl, :], ostage[:])

    nc.compile()
    return nc


def _rope_tables(token_positions):
    pos = np.asarray(token_positions).astype(np.float32)
    half = D_K // 2
    inv_freq = (THETA ** (-np.arange(half, dtype=np.float32) * 2.0 / D_K))
    ang = pos[None, :].astype(np.float32) * inv_freq[:, None]     # [32, S]
    cos = np.cos(ang).astype(np.float32)
    sin = np.sin(ang).astype(np.float32)
    cos128 = np.tile(cos, (4, 1))                                 # [128, S]
    sin128 = np.empty((128, pos.shape[0]), np.float32)
    for g in range(4):
        sgn = -1.0 if (g % 2 == 0) else 1.0
        sin128[g * 32:(g + 1) * 32] = sgn * sin
    return np.ascontiguousarray(cos128), np.ascontiguousarray(sin128)


def kernel(x, W_qkv, W_o, token_positions):
    out, _ = _kernel_impl(x, W_qkv, W_o, token_positions, trace=False)
    return out


def _kernel_impl(x, W_qkv, W_o, token_positions, trace=False):
    global _compiled
    from concourse.bass_utils import run_bass_kernel_spmd

    x = np.asarray(x, dtype=np.float32)
    W_qkv = np.asarray(W_qkv, dtype=np.float32)
    W_o = np.asarray(W_o, dtype=np.float32)

    if _compiled is None:
        _compiled = _build_program()
    nc = _compiled

    cos128, sin128 = _rope_tables(token_positions)
    perm = np.concatenate([np.arange(0, D_K, 2), np.arange(1, D_K, 2)])

    in_maps = []
    for c in range(N_CORES):
        b, g = divmod(c, 2)
        heads = range(g * HPC, (g + 1) * HPC)
        qrows = np.concatenate(
            [W_qkv[h * D_K:(h + 1) * D_K][perm] for h in heads])
        krows = np.concatenate(
            [W_qkv[D + h * D_K:D + (h + 1) * D_K][perm] for h in heads])
        vrows = np.concatenate(
            [W_qkv[2 * D + h * D_K:2 * D + (h + 1) * D_K] for h in heads])
        wqkvt = np.ascontiguousarray(
            np.concatenate([qrows, krows, vrows]).T)              # [1024,1536]
        wot = np.ascontiguousarray(W_o[:, g * DG:(g + 1) * DG].T)  # [512,1024]
        in_maps.append({
            "xt": np.ascontiguousarray(x[b].T),
            "wqkvt": wqkvt,
            "wot": wot,
            "cost": cos128,
            "sint": sin128,
        })

    res = run_bass_kernel_spmd(nc, in_maps, list(range(N_CORES)), trace=trace)
    out = np.empty((BS, S, D), dtype=np.float32)
    for b in range(BS):
        out[b] = res.results[2 * b]["out"] + res.results[2 * b + 1]["out"]
    return out, res.exec_time_ns


# revision 19
# speedup vs baseline: 1.5407x; 1.5407x over previous
"""Multi-head causal attention with interleaved RoPE on 8 Trainium2 cores.

nn_MultiHeadAttention: x[4,2048,1024], W_qkv[3072,1024], W_o[1024,1024],
16 heads x d_k=64, interleaved RoPE, causal softmax.

Sharding: core c = 2*b + g handles batch b (of 4) and head-group g (of 2,
8 heads each). Each core computes a full-width partial output for its batch
(o_heads @ W_o[:, group-cols]); the host sums the two partials per batch
(the "all-reduce after o_proj", done on host at gather time).

Device strategy (per core):
 - host passes x[b] transposed (xT [1024,2048]) and W slices transposed, with
   q/k rows permuted even-first so interleaved RoPE becomes rotate-half.
 - fp32r matmuls everywhere (1 cyc/row vs 4 for fp32 at moving dim >= 256).
 - QKV proj on PE, out q^T/k^T in [head_dim, seq] layout; RoPE applied with
   a gpsimd 32-row swap + DVE mul/mul/add against host-built cos/sin tables
   (sign of sin baked into the table rows).
 - scores computed transposed: S^T[k,q] = k_rot . q_rot per head; exp on ACT
   (1/sqrt(dk) fused into the activation scale; no max-subtraction needed:
   scores are O(15) max, fp32-safe); causal = block skipping + one additive
   -1e30 mask on the 128x128 diagonal block before exp.
 - PV with lhsT = [v | ones]: the softmax denominator falls out as row 64 of
   the PSUM accumulator; normalize after PV, directly producing o^T which is
   exactly the lhsT that o_proj needs. No transposes of P or o anywhere.
"""

import numpy as np
from contextlib import ExitStack

NUM_HEADS = 16
D_K = 64
THETA = 10000.0
BS, S, D = 4, 2048, 1024
N_CORES = 8
HPC = NUM_HEADS // 2          # heads per core = 8
DG = HPC * D_K                # per-core head width = 512
QT2 = 1024                    # q tile (2 PSUM banks)

USE_F32R = True

_compiled = None


def _build_program(stop_after=None):
    import concourse.bass as bass
    import concourse.mybir as mybir
    import concourse.tile as tile
    from concourse import bacc

    F32 = mybir.dt.float32
    FR = mybir.dt.float32r if USE_F32R else mybir.dt.float32
    AF = mybir.ActivationFunctionType

    nc = bacc.Bacc("TRN2", target_bir_lowering=False, debug=False,
                   num_devices=N_CORES)

    xt_d = nc.dram_tensor("xt", [D, S], FR, kind="ExternalInput")
    wqkvt_d = nc.dram_tensor("wqkvt", [D, 3 * DG], FR, kind="ExternalInput")
    wot_d = nc.dram_tensor("wot", [DG, D], FR, kind="ExternalInput")
    cos_d = nc.dram_tensor("cost", [128, S], F32, kind="ExternalInput")
    sin_d = nc.dram_tensor("sint", [128, S], F32, kind="ExternalInput")
    out_d = nc.dram_tensor("out", [S, D], F32, kind="ExternalOutput")

    n_sb = S // 128           # 16 s-blocks
    n_st = S // 512           # 4 s-tiles
    n_db = D // 128           # 8 d-blocks
    inv_sqrt_dk = 1.0 / float(np.sqrt(D_K))

    def load_xt_strip(pool, st):
        """One gpsimd DMA (fp32 -> fp32r cast): [128, 8*512] strip tile where
        cols db*512:(db+1)*512 hold xT[db*128:(db+1)*128, st*512:(st+1)*512]."""
        t = pool.tile([128, n_db * 512], FR, tag="xts", name="xts")
        for db in range(n_db):
            nc.sync.dma_start(
                t[:, db * 512:(db + 1) * 512],
                xt_d.ap()[db * 128:(db + 1) * 128,
                          st * 512:(st + 1) * 512])
        return t

    with tile.TileContext(nc) as tc, ExitStack() as octx:
        OP = octx.enter_context
        # ---------- persistent pools (whole kernel) ----------
        qk_p = OP(tc.tile_pool(name="qk", bufs=1))

        # q_rot/k_rot: [512 e, 2048 s] as 4 tiles of [128, S] each
        qrot = [qk_p.tile([128, S], FR, tag=f"qrot{i}", name=f"qrot{i}")
                for i in range(4)]
        krot = [qk_p.tile([128, S], FR, tag=f"krot{i}", name=f"krot{i}")
                for i in range(4)]
        wot_p = OP(tc.tile_pool(name="wot", bufs=1))
        wot = [wot_p.tile([128, D], FR, tag=f"wot{i}", name=f"wott{i}")
               for i in range(4)]
        const_p = OP(tc.tile_pool(name="amisc", bufs=1))
        # multiplicative causal mask for the S^T diagonal block:
        # 1 where k <= q, 0 where k > q
        dmask = const_p.tile([128, 128], F32)
        nc.gpsimd.memset(dmask[:], 1.0)
        nc.gpsimd.affine_select(
            out=dmask[:], in_=dmask[:],
            compare_op=mybir.AluOpType.is_ge, fill=0.0, base=0,
            pattern=[[1, 128]], channel_multiplier=-1,
        )

        # ============ Phase P1: q/k projection + RoPE ============
        with ExitStack() as ctx:
            P = ctx.enter_context
            cs_p = P(tc.tile_pool(name="cs", bufs=1))
            xt_p = P(tc.tile_pool(name="xtp", bufs=2))
            w_p = P(tc.tile_pool(name="w", bufs=1))
            tmp_p = P(tc.tile_pool(name="tmp", bufs=3))
            rot_p = P(tc.tile_pool(name="rot", bufs=3))
            pp1 = P(tc.tile_pool(name="pp1", bufs=6, space="PSUM"))

            # W_qk^T resident: 8 tiles [128 d, 1024 e]
            wqk = [w_p.tile([128, 2 * DG], FR, tag=f"wqk{i}", name=f"wqk{i}")
                   for i in range(n_db)]
            for db in range(n_db):
                nc.sync.dma_start(
                    wqk[db][:], wqkvt_d.ap()[db * 128:(db + 1) * 128, 0:1024])
            cos_t = cs_p.tile([128, S], F32)
            sin_t = cs_p.tile([128, S], F32)
            nc.scalar.dma_start(cos_t[:], cos_d.ap())
            nc.scalar.dma_start(sin_t[:], sin_d.ap())

            for st in range(n_st):
                xts = load_xt_strip(xt_p, st)
                if st == 1:
                    for t in range(4):
                        nc.scalar.dma_start(
                            wot[t][:], wot_d.ap()[t * 128:(t + 1) * 128, :])
                sl = slice(st * 512, (st + 1) * 512)
                for eb in range(8):
                    dst = qrot[eb] if eb < 4 else krot[eb - 4]
                    esl = slice(eb * 128, (eb + 1) * 128)
                    ps = pp1.tile([128, 512], F32, tag="pp")
                    for db in range(n_db):
                        nc.tensor.matmul(
                            ps[:], wqk[db][:, esl],
                            xts[:, db * 512:(db + 1) * 512],
                            start=(db == 0), stop=(db == n_db - 1))
                    # RoPE: dst = ps*cos + swap32(ps)*sin_signed
                    t1 = rot_p.tile([128, 512], F32, tag="t1")
                    nc.vector.tensor_mul(t1[:], ps[:], cos_t[:, sl])
                    qtmp = tmp_p.tile([128, 512], F32, tag="qtmp")
                    nc.scalar.copy(qtmp[:], ps[:])
                    qsw = tmp_p.tile([128, 512], F32, tag="qsw")
                    for g in range(2):
                        a, b = g * 64, g * 64 + 32
                        nc.scalar.dma_start(qsw[a:a + 32, :], qtmp[b:b + 32, :])
                        nc.sync.dma_start(qsw[b:b + 32, :], qtmp[a:a + 32, :])
                    t2 = rot_p.tile([128, 512], F32, tag="t2")
                    nc.vector.tensor_mul(t2[:], qsw[:], sin_t[:, sl])
                    nc.vector.tensor_add(dst[:, sl], t1[:], t2[:])

        if stop_after == "p1":
            with ExitStack() as ctx:
                op_ = ctx.enter_context(tc.tile_pool(name="dumo", bufs=1))
                for i in range(4):
                    nc.sync.dma_start(
                        out_d.ap()[i * 128:(i + 1) * 128, :],
                        qrot[i][:, 0:D].bitcast(mybir.dt.float32))
            nc.compile()
            return nc

        # ============ Phase P2: v projection into [v | ones] ============
        vaug_p = OP(tc.tile_pool(name="vaug", bufs=1))
        vaug = [vaug_p.tile([128, HPC * (D_K + 1)], FR, tag=f"va{i}",
                        name=f"va{i}") for i in range(n_sb)]
        with ExitStack() as ctx:
            P = ctx.enter_context
            xt_p = P(tc.tile_pool(name="xtp2", bufs=2))
            wv_p = P(tc.tile_pool(name="wv", bufs=1))
            pp2 = P(tc.tile_pool(name="pp2", bufs=6, space="PSUM"))
            ones_t = wv_p.tile([128, HPC], F32, name="ones_t")
            nc.gpsimd.memset(ones_t[:], 1.0)
            wv = [wv_p.tile([128, DG], FR, tag=f"wv{i}", name=f"wv{i}")
                  for i in range(n_db)]
            for db in range(n_db):
                nc.sync.dma_start(
                    wv[db][:],
                    wqkvt_d.ap()[db * 128:(db + 1) * 128, 1024:1536])
            for st in range(n_st):
                xts = load_xt_strip(xt_p, st)
                for j in range(4):
                    sb = st * 4 + j
                    ps = pp2.tile([128, 512], F32, tag="pp")
                    for db in range(n_db):
                        nc.tensor.matmul(
                            ps[:],
                            xts[:, db * 512 + j * 128:db * 512 + (j + 1) * 128],
                            wv[db][:],
                            start=(db == 0), stop=(db == n_db - 1))
                    src = ps[:].rearrange("p (h c) -> p h c", c=D_K)
                    dst = vaug[sb][:].rearrange("p (h c) -> p h c", c=D_K + 1)
                    nc.vector.tensor_copy(dst[:, :, 0:D_K], src)
                    nc.vector.tensor_copy(
                        dst[:, :, D_K:D_K + 1],
                        ones_t[:].rearrange("p (h c) -> p h c", c=1))

        if stop_after == "p2":
            for i in range(4):
                nc.sync.dma_start(
                    out_d.ap()[i * 128:(i + 1) * 128, :],
                    vaug[i][:, 0:520].bitcast(mybir.dt.float32)[:, 0:512].rearrange("p n -> p n"))
            nc.compile()
            return nc

        # ============ Phase A: attention ============
        ot_p = OP(tc.tile_pool(name="ot", bufs=1))
        ot = [ot_p.tile([128, S], FR, tag=f"ot{i}", name=f"oti{i}")
              for i in range(4)]

        with ExitStack() as ctx:
            P = ctx.enter_context
            pt_p = P(tc.tile_pool(name="pt", bufs=3))
            nrm_p = P(tc.tile_pool(name="nrm", bufs=2))
            sps_p = P(tc.tile_pool(name="sps", bufs=2, space="PSUM"))
            ops_p = P(tc.tile_pool(name="ops", bufs=2, space="PSUM"))

            for h in range(HPC):
                ti, po = h // 2, (h % 2) * 64
                vlo = h * (D_K + 1)
                for q2 in range(S // QT2):
                    q0 = q2 * QT2
                    kb_end = (q0 + QT2) // 128
                    kb_last0 = q0 // 128 + 3      # last kb writing bank 0
                    ops = ops_p.tile([D_K + 1, QT2], F32, tag="ot")
                    for kb in range(kb_end):
                        c0 = max(0, kb * 128 - q0)
                        sc = sps_p.tile([128, QT2], F32, tag="sc")
                        # scores S^T (k on partitions, q on free), per bank
                        if c0 < 512:
                            nc.tensor.matmul(
                                sc[:, c0:512],
                                krot[ti][po:po + 64, kb * 128:(kb + 1) * 128],
                                qrot[ti][po:po + 64, q0 + c0:q0 + 512],
                                start=True, stop=True)
                        b1 = max(c0, 512)
                        nc.tensor.matmul(
                            sc[:, b1:QT2],
                            krot[ti][po:po + 64, kb * 128:(kb + 1) * 128],
                            qrot[ti][po:po + 64, q0 + b1:q0 + QT2],
                            start=True, stop=True)
                        pt = pt_p.tile([128, QT2], FR, tag="pt")
                        nc.scalar.activation(pt[:, c0:QT2], sc[:, c0:QT2],
                                             AF.Exp, scale=inv_sqrt_dk)
                        # causal diagonal: multiplicative post-exp (SBUF 2x)
                        if kb * 128 >= q0:
                            nc.vector.tensor_mul(pt[:, c0:c0 + 128],
                                                 pt[:, c0:c0 + 128], dmask[:])
                        # PV (+ ones row -> denominator lands in row 64)
                        if c0 < 512:
                            nc.tensor.matmul(
                                ops[:, c0:512],
                                vaug[kb][:, vlo:vlo + D_K + 1],
                                pt[:, c0:512],
                                start=(kb == 0), stop=(kb == kb_last0))
                        nc.tensor.matmul(
                            ops[:, b1:QT2],
                            vaug[kb][:, vlo:vlo + D_K + 1],
                            pt[:, b1:QT2],
                            start=(kb == 0), stop=(kb == kb_end - 1))
                    # normalize: o^T[dv, q] * (1/den[q]); the reciprocal
                    # reads PSUM partition 64 directly (cross-partition
                    # single-partition DVE read, HW-verified)
                    rinv = nrm_p.tile([1, QT2], F32, tag="rinv")
                    nc.vector.reciprocal(rinv[:], ops[D_K:D_K + 1, :])
                    den = nrm_p.tile([64, QT2], F32, tag="den")
                    nc.gpsimd.partition_broadcast(den[:], rinv[:])
                    if po == 0:
                        nc.vector.tensor_mul(ot[ti][0:64, q0:q0 + QT2],
                                             ops[0:D_K, :], den[:])
                    else:
                        onrm = nrm_p.tile([64, QT2], FR, tag="onrm")
                        nc.vector.tensor_mul(onrm[:], ops[0:D_K, :], den[:])
                        nc.sync.dma_start(ot[ti][64:128, q0:q0 + QT2],
                                          onrm[:])

            # ============ o_proj (same scope: reuses the sc PSUM slots) ====
            outs_p = P(tc.tile_pool(name="outs", bufs=2))
            for sb in range(n_sb):
                ssl = slice(sb * 128, (sb + 1) * 128)
                ostage = outs_p.tile([128, D], F32, tag="ostage")
                for eh in range(2):
                    esl = slice(eh * 512, (eh + 1) * 512)
                    ps = sps_p.tile([128, 512], F32, tag="sc")
                    for t in range(4):
                        nc.tensor.matmul(ps[:], ot[t][:, ssl], wot[t][:, esl],
                                         start=(t == 0), stop=(t == 3))
                    nc.scalar.copy(ostage[:, esl], ps[:])
                nc.sync.dma_start(out_d.ap()[ssl, :], ostage[:])

    nc.compile()
    return nc


def _rope_tables(token_positions):
    pos = np.asarray(token_positions).astype(np.float32)
    half = D_K // 2
    inv_freq = (THETA ** (-np.arange(half, dtype=np.float32) * 2.0 / D_K))
    ang = pos[None, :].astype(np.float32) * inv_freq[:, None]     # [32, S]
    cos = np.cos(ang).astype(np.float32)
    sin = np.sin(ang).astype(np.float32)
    cos128 = np.tile(cos, (4, 1))                                 # [128, S]
    sin128 = np.empty((128, pos.shape[0]), np.float32)
    for g in range(4):
        sgn = -1.0 if (g % 2 == 0) else 1.0
        sin128[g * 32:(g + 1) * 32] = sgn * sin
    return np.ascontiguousarray(cos128), np.ascontiguousarray(sin128)


def kernel(x, W_qkv, W_o, token_positions):
    out, _ = _kernel_impl(x, W_qkv, W_o, token_positions, trace=False)
    return out


def _kernel_impl(x, W_qkv, W_o, token_positions, trace=False):
    global _compiled
    from concourse.bass_utils import run_bass_kernel_spmd

    x = np.asarray(x, dtype=np.float32)
    W_qkv = np.asarray(W_qkv, dtype=np.float32)
    W_o = np.asarray(W_o, dtype=np.float32)

    if _compiled is None:
        _compiled = _build_program()
    nc = _compiled

    cos128, sin128 = _rope_tables(token_positions)
    perm = np.concatenate([np.arange(0, D_K, 2), np.arange(1, D_K, 2)])

    in_maps = []
    for c in range(N_CORES):
        b, g = divmod(c, 2)
        heads = range(g * HPC, (g + 1) * HPC)
        qrows = np.concatenate(
            [W_qkv[h * D_K:(h + 1) * D_K][perm] for h in heads])
        krows = np.concatenate(
            [W_qkv[D + h * D_K:D + (h + 1) * D_K][perm] for h in heads])
        vrows = np.concatenate(
            [W_qkv[2 * D + h * D_K:2 * D + (h + 1) * D_K] for h in heads])
        wqkvt = np.ascontiguousarray(
            np.concatenate([qrows, krows, vrows]).T)              # [1024,1536]
        wot = np.ascontiguousarray(W_o[:, g * DG:(g + 1) * DG].T)  # [512,1024]
        in_maps.append({
            "xt": np.ascontiguousarray(x[b].T),
            "wqkvt": wqkvt,
            "wot": wot,
            "cost": cos128,
            "sint": sin128,
        })

    res = run_bass_kernel_spmd(nc, in_maps, list(range(N_CORES)), trace=trace)
    out = np.empty((BS, S, D), dtype=np.float32)
    for b in range(BS):
        out[b] = res.results[2 * b]["out"] + res.results[2 * b + 1]["out"]
    return out, res.exec_time_ns


# revision 31
# speedup vs baseline: 16.5541x; 10.7446x over previous
"""Multi-head causal attention with interleaved RoPE on 8 Trainium2 cores.

nn_MultiHeadAttention: x[4,2048,1024], W_qkv[3072,1024], W_o[1024,1024],
16 heads x d_k=64, interleaved RoPE, causal softmax.

Sharding: core c = 2*b + g handles batch b (of 4) and head-group g (of 2,
8 heads each). Each core computes a full-width partial output for its batch
(o_heads @ W_o[:, group-cols]); the host sums the two partials per batch
(the "all-reduce after o_proj", done on host at gather time).

Device strategy (per core):
 - host passes x[b] transposed (xT [1024,2048]) and W slices transposed, with
   q/k rows permuted even-first so interleaved RoPE becomes rotate-half.
 - fp32r matmuls everywhere (1 cyc/row vs 4 for fp32 at moving dim >= 256).
 - QKV proj on PE, out q^T/k^T in [head_dim, seq] layout; RoPE applied with
   a gpsimd 32-row swap + DVE mul/mul/add against host-built cos/sin tables
   (sign of sin baked into the table rows).
 - scores computed transposed: S^T[k,q] = k_rot . q_rot per head; exp on ACT
   (1/sqrt(dk) fused into the activation scale; no max-subtraction needed:
   scores are O(15) max, fp32-safe); causal = block skipping + one additive
   -1e30 mask on the 128x128 diagonal block before exp.
 - PV with lhsT = [v | ones]: the softmax denominator falls out as row 64 of
   the PSUM accumulator; normalize after PV, directly producing o^T which is
   exactly the lhsT that o_proj needs. No transposes of P or o anywhere.
"""

import numpy as np
from contextlib import ExitStack

NUM_HEADS = 16
D_K = 64
THETA = 10000.0
BS, S, D = 4, 2048, 1024
N_CORES = 8
HPC = NUM_HEADS // 2          # heads per core = 8
DG = HPC * D_K                # per-core head width = 512
QT2 = 1024                    # q tile (2 PSUM banks)

USE_F32R = True

_compiled = None


def _build_program(stop_after=None):
    import concourse.bass as bass
    import concourse.mybir as mybir
    import concourse.tile as tile
    from concourse import bacc

    F32 = mybir.dt.float32
    FR = mybir.dt.float32r if USE_F32R else mybir.dt.float32
    AF = mybir.ActivationFunctionType

    nc = bacc.Bacc("TRN2", target_bir_lowering=False, debug=False,
                   num_devices=N_CORES)

    xt_d = nc.dram_tensor("xt", [D, S], FR, kind="ExternalInput")
    wqkvt_d = nc.dram_tensor("wqkvt", [D, 3 * DG], FR, kind="ExternalInput")
    wot_d = nc.dram_tensor("wot", [DG, D], FR, kind="ExternalInput")
    perm_d = nc.dram_tensor("perm", [128, 128], FR, kind="ExternalInput")
    cos_d = nc.dram_tensor("cost", [128, S], F32, kind="ExternalInput")
    sin_d = nc.dram_tensor("sint", [128, S], F32, kind="ExternalInput")
    out_d = nc.dram_tensor("out", [S, D], F32, kind="ExternalOutput")

    n_sb = S // 128           # 16 s-blocks
    n_st = S // 512           # 4 s-tiles
    n_db = D // 128           # 8 d-blocks
    inv_sqrt_dk = 1.0 / float(np.sqrt(D_K))

    def load_xt_strip(pool, st):
        """One gpsimd DMA (fp32 -> fp32r cast): [128, 8*512] strip tile where
        cols db*512:(db+1)*512 hold xT[db*128:(db+1)*128, st*512:(st+1)*512]."""
        t = pool.tile([128, n_db * 512], FR, tag="xts", name="xts")
        src = xt_d.ap().rearrange("(db p) (st s) -> p db (st s)",
                                  p=128, st=n_st)
        nc.sync.dma_start(t[:].rearrange("p (db s) -> p db s", db=n_db),
                          src[:, :, st * 512:(st + 1) * 512])
        return t

    with tile.TileContext(nc) as tc, ExitStack() as octx:
        OP = octx.enter_context
        # ---------- persistent pools (whole kernel) ----------
        qk_p = OP(tc.tile_pool(name="qk", bufs=1))

        # q_rot/k_rot: [512 e, 2048 s] as 4 tiles of [128, S] each
        qrot = [qk_p.tile([128, S], FR, tag=f"qrot{i}", name=f"qrot{i}")
                for i in range(4)]
        krot = [qk_p.tile([128, S], FR, tag=f"krot{i}", name=f"krot{i}")
                for i in range(4)]
        wot_p = OP(tc.tile_pool(name="wot", bufs=1))
        wot = [wot_p.tile([128, D], FR, tag=f"wot{i}", name=f"wott{i}")
               for i in range(4)]
        const_p = OP(tc.tile_pool(name="amisc", bufs=1))
        # multiplicative causal mask for the S^T diagonal block:
        # 1 where k <= q, 0 where k > q
        dmask = const_p.tile([128, 128], F32)
        nc.gpsimd.memset(dmask[:], 1.0)
        nc.gpsimd.affine_select(
            out=dmask[:], in_=dmask[:],
            compare_op=mybir.AluOpType.is_ge, fill=0.0, base=0,
            pattern=[[1, 128]], channel_multiplier=-1,
        )

        # ============ Phase P1: q/k projection + RoPE ============
        with ExitStack() as ctx:
            P = ctx.enter_context
            cs_p = P(tc.tile_pool(name="cs", bufs=1))
            xt_p = P(tc.tile_pool(name="xtp", bufs=2))
            w_p = P(tc.tile_pool(name="w", bufs=1))
            tmp_p = P(tc.tile_pool(name="tmp", bufs=5))
            rot_p = P(tc.tile_pool(name="rot", bufs=4))
            pp1 = P(tc.tile_pool(name="pp1", bufs=8, space="PSUM"))

            # strip 0 first: the very first matmul needs it
            xts_next = load_xt_strip(xt_p, 0)
            # W_qk^T resident as separate lo/hi tiles (e 0..511 / 512..1023)
            # so early eb blocks depend only on the lo DMAs
            wqk_lo = [w_p.tile([128, DG], FR, tag=f"wqkl{i}", name=f"wqkl{i}")
                      for i in range(n_db)]
            wqk_hi = [w_p.tile([128, DG], FR, tag=f"wqkh{i}", name=f"wqkh{i}")
                      for i in range(n_db)]
            for db in range(n_db):
                nc.sync.dma_start(
                    wqk_lo[db][:],
                    wqkvt_d.ap()[db * 128:(db + 1) * 128, 0:512])
            perm_t = cs_p.tile([128, 128], FR, name="perm_t")
            nc.scalar.dma_start(perm_t[:], perm_d.ap())
            cos_t = cs_p.tile([128, S], F32)
            sin_t = cs_p.tile([128, S], F32)
            nc.scalar.dma_start(cos_t[:], cos_d.ap())
            nc.scalar.dma_start(sin_t[:], sin_d.ap())
            for db in range(n_db):
                nc.sync.dma_start(
                    wqk_hi[db][:],
                    wqkvt_d.ap()[db * 128:(db + 1) * 128, 512:1024])

            def wqk_slice(db, eb):
                if eb < 4:
                    return wqk_lo[db][:, eb * 128:(eb + 1) * 128]
                return wqk_hi[db][:, (eb - 4) * 128:(eb - 3) * 128]

            def rope_phase2(state):
                """swap-matmul + t2 + add for a previous block (lag-1 so the
                perm matmul does not head-of-line-block the PE queue)."""
                qtmp, t1, dst, sl = state
                psw = pp1.tile([128, 512], F32, tag="pp", name="psw")
                nc.tensor.matmul(psw[:], perm_t[:], qtmp[:],
                                 start=True, stop=True)
                t2 = rot_p.tile([128, 512], F32, tag="t2", name="t2")
                nc.vector.tensor_mul(t2[:], psw[:], sin_t[:, sl])
                nc.vector.tensor_add(dst[:, sl], t1[:], t2[:])

            pending = None
            for st in range(n_st):
                xts = xts_next
                if st + 1 < n_st:
                    xts_next = load_xt_strip(xt_p, st + 1)
                if st == 1:
                    for t in range(4):
                        nc.scalar.dma_start(
                            wot[t][:], wot_d.ap()[t * 128:(t + 1) * 128, :])
                sl = slice(st * 512, (st + 1) * 512)
                for eb in range(8):
                    dst = qrot[eb] if eb < 4 else krot[eb - 4]
                    ps = pp1.tile([128, 512], F32, tag="pp")
                    for db in range(n_db):
                        nc.tensor.matmul(
                            ps[:], wqk_slice(db, eb),
                            xts[:, db * 512:(db + 1) * 512],
                            start=(db == 0), stop=(db == n_db - 1))
                    qtmp = tmp_p.tile([128, 512], FR, tag="qtmp")
                    nc.scalar.copy(qtmp[:], ps[:])
                    t1 = rot_p.tile([128, 512], F32, tag="t1")
                    nc.vector.tensor_mul(t1[:], qtmp[:], cos_t[:, sl])
                    if pending is not None:
                        rope_phase2(pending)
                    pending = (qtmp, t1, dst, sl)
            rope_phase2(pending)

        if stop_after == "p1":
            with ExitStack() as ctx:
                op_ = ctx.enter_context(tc.tile_pool(name="dumo", bufs=1))
                for i in range(4):
                    nc.sync.dma_start(
                        out_d.ap()[i * 128:(i + 1) * 128, :],
                        qrot[i][:, 0:D].bitcast(mybir.dt.float32))
            nc.compile()
            return nc

        # ============ Phase P2: v projection into [v | ones] ============
        vaug_p = OP(tc.tile_pool(name="vaug", bufs=1))
        vaug = [vaug_p.tile([128, HPC * (D_K + 1)], FR, tag=f"va{i}",
                        name=f"va{i}") for i in range(n_sb)]
        with ExitStack() as ctx:
            P = ctx.enter_context
            xt_p = P(tc.tile_pool(name="xtp2", bufs=2))
            wv_p = P(tc.tile_pool(name="wv", bufs=1))
            pp2 = P(tc.tile_pool(name="pp2", bufs=6, space="PSUM"))
            ones_t = wv_p.tile([128, HPC], F32, name="ones_t")
            nc.gpsimd.memset(ones_t[:], 1.0)
            wv = [wv_p.tile([128, DG], FR, tag=f"wv{i}", name=f"wv{i}")
                  for i in range(n_db)]
            for db in range(n_db):
                nc.sync.dma_start(
                    wv[db][:],
                    wqkvt_d.ap()[db * 128:(db + 1) * 128, 1024:1536])
            for st in range(n_st):
                xts = load_xt_strip(xt_p, st)
                for j in range(4):
                    sb = st * 4 + j
                    ps = pp2.tile([128, 512], F32, tag="pp")
                    for db in range(n_db):
                        nc.tensor.matmul(
                            ps[:],
                            xts[:, db * 512 + j * 128:db * 512 + (j + 1) * 128],
                            wv[db][:],
                            start=(db == 0), stop=(db == n_db - 1))
                    src = ps[:].rearrange("p (h c) -> p h c", c=D_K)
                    dst = vaug[sb][:].rearrange("p (h c) -> p h c", c=D_K + 1)
                    nc.vector.tensor_copy(dst[:, :, 0:D_K], src)
                    nc.vector.tensor_copy(
                        dst[:, :, D_K:D_K + 1],
                        ones_t[:].rearrange("p (h c) -> p h c", c=1))

        if stop_after == "p2":
            for i in range(4):
                nc.sync.dma_start(
                    out_d.ap()[i * 128:(i + 1) * 128, :],
                    vaug[i][:, 0:520].bitcast(mybir.dt.float32)[:, 0:512].rearrange("p n -> p n"))
            nc.compile()
            return nc

        # ============ Phase A: attention ============
        ot_p = OP(tc.tile_pool(name="ot", bufs=1))
        ot = [ot_p.tile([128, S], FR, tag=f"ot{i}", name=f"oti{i}")
              for i in range(4)]

        with ExitStack() as ctx:
            P = ctx.enter_context
            pt_p = P(tc.tile_pool(name="pt", bufs=3))
            nrm_p = P(tc.tile_pool(name="nrm", bufs=3))
            sps_p = P(tc.tile_pool(name="sps", bufs=2, space="PSUM"))
            ops_p = P(tc.tile_pool(name="ops", bufs=2, space="PSUM"))

            outs_p = P(tc.tile_pool(name="outs", bufs=2))
            for q2 in range(S // QT2):
                for h in range(HPC):
                    ti, po = h // 2, (h % 2) * 64
                    vlo = h * (D_K + 1)
                    q0 = q2 * QT2
                    kb_end = (q0 + QT2) // 128
                    kb_last0 = q0 // 128 + 3      # last kb writing bank 0
                    ops = ops_p.tile([D_K + 1, QT2], F32, tag="ot")
                    def emit_pv(kb, pt):
                        c0 = max(0, kb * 128 - q0)
                        b1 = max(c0, 512)
                        if c0 < 512:
                            nc.tensor.matmul(
                                ops[:, c0:512],
                                vaug[kb][:, vlo:vlo + D_K + 1],
                                pt[:, c0:512],
                                start=(kb == 0), stop=(kb == kb_last0))
                        nc.tensor.matmul(
                            ops[:, b1:QT2],
                            vaug[kb][:, vlo:vlo + D_K + 1],
                            pt[:, b1:QT2],
                            start=(kb == 0), stop=(kb == kb_end - 1))

                    pend_pv = None
                    for kb in range(kb_end):
                        c0 = max(0, kb * 128 - q0)
                        sc = sps_p.tile([128, QT2], F32, tag="sc")
                        # scores S^T (k on partitions, q on free), per bank
                        if c0 < 512:
                            nc.tensor.matmul(
                                sc[:, c0:512],
                                krot[ti][po:po + 64, kb * 128:(kb + 1) * 128],
                                qrot[ti][po:po + 64, q0 + c0:q0 + 512],
                                start=True, stop=True)
                        b1 = max(c0, 512)
                        nc.tensor.matmul(
                            sc[:, b1:QT2],
                            krot[ti][po:po + 64, kb * 128:(kb + 1) * 128],
                            qrot[ti][po:po + 64, q0 + b1:q0 + QT2],
                            start=True, stop=True)
                        pt = pt_p.tile([128, QT2], FR, tag="pt")
                        nc.scalar.activation(pt[:, c0:QT2], sc[:, c0:QT2],
                                             AF.Exp, scale=inv_sqrt_dk)
                        # causal diagonal: multiplicative post-exp (SBUF 2x)
                        if kb * 128 >= q0:
                            nc.vector.tensor_mul(pt[:, c0:c0 + 128],
                                                 pt[:, c0:c0 + 128], dmask[:])
                        if pend_pv is not None:
                            emit_pv(*pend_pv)
                        pend_pv = (kb, pt)
                    emit_pv(*pend_pv)
                    # normalize: o^T[dv, q] * (1/den[q]); the reciprocal
                    # reads PSUM partition 64 directly (cross-partition
                    # single-partition DVE read, HW-verified)
                    rinv = nrm_p.tile([1, QT2], F32, tag="rinv")
                    nc.vector.reciprocal(rinv[:], ops[D_K:D_K + 1, :])
                    den = nrm_p.tile([64, QT2], F32, tag="den")
                    nc.gpsimd.partition_broadcast(den[:], rinv[:])
                    if po == 0:
                        nc.vector.tensor_mul(ot[ti][0:64, q0:q0 + QT2],
                                             ops[0:D_K, :], den[:])
                    else:
                        onrm = nrm_p.tile([64, QT2], FR, tag="onrm")
                        nc.vector.tensor_mul(onrm[:], ops[0:D_K, :], den[:])
                        nc.sync.dma_start(ot[ti][64:128, q0:q0 + QT2],
                                          onrm[:])

            # o_proj tail (reuses the sc PSUM slots)
            if True:
                for sb in range(n_sb):
                    ssl = slice(sb * 128, (sb + 1) * 128)
                    ostage = outs_p.tile([128, D], F32, tag="ostage")
                    for eh in range(2):
                        esl = slice(eh * 512, (eh + 1) * 512)
                        ps = sps_p.tile([128, 512], F32, tag="sc")
                        for t in range(4):
                            nc.tensor.matmul(ps[:], ot[t][:, ssl],
                                             wot[t][:, esl],
                                             start=(t == 0), stop=(t == 3))
                        if eh == 0:
                            nc.vector.tensor_copy(ostage[:, esl], ps[:])
                        else:
                            nc.scalar.copy(ostage[:, esl], ps[:])
                    nc.sync.dma_start(out_d.ap()[ssl, :], ostage[:])

    nc.compile()
    return nc


def _perm128():
    """[128,128] fp32 permutation: out = P.T @ x swaps 32-row halves within
    each 64-row group. P[k, m] = 1 iff k == swap(m)."""
    p = np.zeros((128, 128), np.float32)
    for m in range(128):
        k = m + 32 if (m % 64) < 32 else m - 32
        p[k, m] = 1.0
    return p


def _rope_tables(token_positions):
    pos = np.asarray(token_positions).astype(np.float32)
    half = D_K // 2
    inv_freq = (THETA ** (-np.arange(half, dtype=np.float32) * 2.0 / D_K))
    ang = pos[None, :].astype(np.float32) * inv_freq[:, None]     # [32, S]
    cos = np.cos(ang).astype(np.float32)
    sin = np.sin(ang).astype(np.float32)
    cos128 = np.tile(cos, (4, 1))                                 # [128, S]
    sin128 = np.empty((128, pos.shape[0]), np.float32)
    for g in range(4):
        sgn = -1.0 if (g % 2 == 0) else 1.0
        sin128[g * 32:(g + 1) * 32] = sgn * sin
    return np.ascontiguousarray(cos128), np.ascontiguousarray(sin128)


def kernel(x, W_qkv, W_o, token_positions):
    out, _ = _kernel_impl(x, W_qkv, W_o, token_positions, trace=False)
    return out


def _kernel_impl(x, W_qkv, W_o, token_positions, trace=False):
    global _compiled
    from concourse.bass_utils import run_bass_kernel_spmd

    x = np.asarray(x, dtype=np.float32)
    W_qkv = np.asarray(W_qkv, dtype=np.float32)
    W_o = np.asarray(W_o, dtype=np.float32)

    if _compiled is None:
        _compiled = _build_program()
    nc = _compiled

    cos128, sin128 = _rope_tables(token_positions)
    perm = np.concatenate([np.arange(0, D_K, 2), np.arange(1, D_K, 2)])

    in_maps = []
    for c in range(N_CORES):
        b, g = divmod(c, 2)
        heads = range(g * HPC, (g + 1) * HPC)
        qrows = np.concatenate(
            [W_qkv[h * D_K:(h + 1) * D_K][perm] for h in heads])
        krows = np.concatenate(
            [W_qkv[D + h * D_K:D + (h + 1) * D_K][perm] for h in heads])
        vrows = np.concatenate(
            [W_qkv[2 * D + h * D_K:2 * D + (h + 1) * D_K] for h in heads])
        wqkvt = np.ascontiguousarray(
            np.concatenate([qrows, krows, vrows]).T)              # [1024,1536]
        wot = np.ascontiguousarray(W_o[:, g * DG:(g + 1) * DG].T)  # [512,1024]
        in_maps.append({
            "xt": np.ascontiguousarray(x[b].T),
            "wqkvt": wqkvt,
            "wot": wot,
            "perm": _perm128(),
            "cost": cos128,
            "sint": sin128,
        })

    res = run_bass_kernel_spmd(nc, in_maps, list(range(N_CORES)), trace=trace)
    out = np.empty((BS, S, D), dtype=np.float32)
    for b in range(BS):
        out[b] = res.results[2 * b]["out"] + res.results[2 * b + 1]["out"]
    return out, res.exec_time_ns
